# revision 12
# baseline (speedup 1.0000x reference)
"""Trainium2 Bass kernel for GQA sliding-window attention with RoPE + soft-cap.

Problem (hardcoded): B=2, T=2048, D=2048, 16 q-heads / 4 kv-heads, head_dim=128,
WINDOW=1024 (causal sliding window), soft-cap 50.

Sharding: 8 cores = 2 batches x 4-way head-split tensor parallel.
Core c handles batch c//4, q-heads [4g:4g+4] and kv-head g where g = c%4.
Each core emits a partial [T, D] output (sum over its 4 heads); the host sums
the 4 TP partials per batch (the TP all-reduce is done in the unshard step).
"""

import sys

sys.path.insert(0, "/opt/trn_rl_repo")

import math

import numpy as np

import concourse.mybir as mybir
import concourse.tile as tile
from concourse import bacc
from concourse.bass_utils import run_bass_kernel_spmd
from concourse.masks import make_identity

# ---------------------------------------------------------------- constants
B, T, D = 2, 2048, 2048
NH, NKV, HD = 16, 4, 128
GQ = NH // NKV  # 4 q-heads per kv head (= heads per core)
WINDOW = 1024
SOFT_CAP = 50.0
P = 128  # partitions
NT = T // P  # 16 row tiles
ND = D // P  # 16 D chunks
MAXW = 1152  # widest band: 8 full tiles + diagonal tile
MASK_VAL = -1e30

FP32 = mybir.dt.float32
FP32R = mybir.dt.float32r
BF16 = mybir.dt.bfloat16

_COMPILED = {}


def _band(i):
    """Key tiles attended by row tile i: j in [max(0, i-8), i]."""
    jfirst = max(0, i - (WINDOW // P))
    return jfirst, i - jfirst + 1  # first j, tile count (<= 9)


def _chunks(w):
    """Split band width w into matmul chunks aligned to 512-col psum banks."""
    out = []
    c0 = 0
    while c0 < w:
        c1 = min(c0 + 512, w)
        out.append((c0, c1))
        c0 = c1
    return out


def build_program():
    nc = bacc.Bacc(None, target_bir_lowering=False, debug=False)

    xt_d = nc.declare_dram_parameter("xt", [D, T], FP32R, isOutput=False)
    wqkv_d = nc.declare_dram_parameter("wqkv", [D, (GQ + 2) * HD], FP32R, isOutput=False)
    wvec_d = nc.declare_dram_parameter("wvec", [GQ, HD, D], BF16, isOutput=False)
    cos_d = nc.declare_dram_parameter("costab", [T, HD], FP32, isOutput=False)
    sin_d = nc.declare_dram_parameter("sintab", [T, HD], FP32, isOutput=False)
    out_d = nc.declare_dram_parameter("out", [T, D], FP32, isOutput=True)

    tanh_scale = 1.0 / (SOFT_CAP * math.sqrt(HD))

    with tile.TileContext(nc) as tc:
        with (
            tc.tile_pool(name="const", bufs=1) as const,
            tc.tile_pool(name="persist", bufs=1) as persist,
        ):
            ident = const.tile([P, P], BF16)
            make_identity(nc, ident)
            # diag mask: valid iff col <= row (causal within diagonal tile)
            maskdiag = const.tile([P, P], FP32)
            nc.gpsimd.memset(maskdiag, 0.0)
            nc.gpsimd.affine_select(
                out=maskdiag,
                in_=maskdiag,
                compare_op=mybir.AluOpType.is_ge,
                fill=MASK_VAL,
                base=0,
                pattern=[[-1, P]],
                channel_multiplier=1,
            )
            # edge mask: valid iff col > row (window cutoff in oldest tile)
            maskedge = const.tile([P, P], FP32)
            nc.gpsimd.memset(maskedge, 0.0)
            nc.gpsimd.affine_select(
                out=maskedge,
                in_=maskedge,
                compare_op=mybir.AluOpType.is_ge,
                fill=MASK_VAL,
                base=-1,
                pattern=[[1, P]],
                channel_multiplier=-1,
            )

            # resident tensors
            wqkv_sb = persist.tile([P, ND, (GQ + 2) * HD], FP32R)
            wqkv_src = wqkv_d[:].rearrange("(c p) w -> p c w", p=P)
            for d in range(ND):
                nc.sync.dma_start(out=wqkv_sb[:, d, :], in_=wqkv_src[:, d, :])
            cos_sb = persist.tile([P, NT, HD], FP32)
            nc.sync.dma_start(out=cos_sb, in_=cos_d[:].rearrange("(c p) h -> p c h", p=P))
            sin_sb = persist.tile([P, NT, HD], FP32)
            nc.sync.dma_start(out=sin_sb, in_=sin_d[:].rearrange("(c p) h -> p c h", p=P))
            wvec_sb = persist.tile([P, GQ, D], BF16)
            nc.sync.dma_start(
                out=wvec_sb,
                in_=wvec_d[:].rearrange("g (p) d -> p g d", p=P),
            )

            # q^T blocks: (ti, n) block of [h=128, t=128] at cols 512*ti+128*n
            qtall = persist.tile([P, NT * GQ * P], BF16)
            kt = persist.tile([P, T], BF16)
            vres = persist.tile([P, T], BF16)
            # enc^T blocks: same (i, n) block layout as qtall
            enctall = persist.tile([P, NT * GQ * P], BF16)

            # transpose-group half selector (packed psum double-buffer)
            tg = [0]

            with (
                tc.tile_pool(name="xa", bufs=2) as xa_pool,
                tc.tile_pool(name="ra", bufs=3) as ra_pool,
                tc.tile_pool(name="tb", bufs=2) as tb_pool,
                tc.tile_pool(name="pb", bufs=2) as pb_pool,
                tc.tile_pool(name="ptb", bufs=3) as ptb_pool,
                tc.tile_pool(name="rb", bufs=2) as rb_pool,
                tc.tile_pool(name="oc", bufs=2) as oc_pool,
                tc.tile_pool(name="pa", bufs=1, space="PSUM") as pa_pool,
                tc.tile_pool(name="ptx", bufs=1, space="PSUM") as ptx_pool,
                tc.tile_pool(name="sb", bufs=1, space="PSUM") as s_pool,
                tc.tile_pool(name="acc", bufs=2, space="PSUM") as acc_pool,
            ):
                # one bank, manually split into two [P, 512] halves
                ptx = ptx_pool.tile([P, 2, 512], BF16, name="ptx")

                def phase_a(ti):
                    tsl = slice(ti * P, (ti + 1) * P)
                    xt_sb = xa_pool.tile([P, ND, P], FP32R, tag="xt", name="xt_sb")
                    nc.sync.dma_start(
                        out=xt_sb,
                        in_=xt_d[:].rearrange("(c p) t -> p c t", p=P)[:, :, tsl],
                    )
                    psq = pa_pool.tile([P, GQ * HD], FP32, tag="psq", name="psq")
                    pskv = pa_pool.tile([P, 2 * HD], FP32, tag="pskv", name="pskv")
                    for d in range(ND):
                        nc.tensor.matmul(
                            psq,
                            lhsT=xt_sb[:, d, :],
                            rhs=wqkv_sb[:, d, 0 : GQ * HD],
                            start=(d == 0),
                            stop=(d == ND - 1),
                        )
                        nc.tensor.matmul(
                            pskv,
                            lhsT=xt_sb[:, d, :],
                            rhs=wqkv_sb[:, d, GQ * HD : (GQ + 2) * HD],
                            start=(d == 0),
                            stop=(d == ND - 1),
                        )

                    # RoPE, batched across heads (tables hold [cos|cos], [-sin|+sin])
                    qr = ra_pool.tile([P, GQ * HD], BF16, tag="qr", name="qr")
                    psq3 = psq.rearrange("p (n h) -> p n h", n=GQ)
                    qr3 = qr.rearrange("p (n h) -> p n h", n=GQ)
                    cosb = cos_sb[:, ti, :].unsqueeze(1).broadcast_to([P, GQ, HD])
                    sinb0 = sin_sb[:, ti, 0:64].unsqueeze(1).broadcast_to([P, GQ, 64])
                    sinb1 = sin_sb[:, ti, 64:128].unsqueeze(1).broadcast_to([P, GQ, 64])
                    tmpc = ra_pool.tile([P, GQ * HD], FP32, tag="tmpc", name="tmpc")
                    tmps = ra_pool.tile([P, GQ * HD], FP32, tag="tmps", name="tmps")
                    tmpc3 = tmpc.rearrange("p (n h) -> p n h", n=GQ)
                    tmps3 = tmps.rearrange("p (n h) -> p n h", n=GQ)
                    nc.vector.tensor_mul(tmpc3, psq3, cosb)
                    nc.vector.tensor_mul(tmps3[:, :, 0:64], psq3[:, :, 64:128], sinb0)
                    nc.vector.tensor_mul(tmps3[:, :, 64:128], psq3[:, :, 0:64], sinb1)
                    nc.vector.tensor_add(qr3, tmpc3, tmps3)
                    kr = ra_pool.tile([P, HD], BF16, tag="kr", name="kr")
                    tmpk = ra_pool.tile([P, HD], FP32, tag="tmpk", name="tmpk")
                    tmpk2 = ra_pool.tile([P, HD], FP32, tag="tmpk2", name="tmpk2")
                    nc.vector.tensor_mul(tmpk, pskv[:, 0:HD], cos_sb[:, ti, :])
                    nc.vector.tensor_mul(
                        tmpk2[:, 0:64], pskv[:, 64:128], sin_sb[:, ti, 0:64]
                    )
                    nc.vector.tensor_mul(
                        tmpk2[:, 64:128], pskv[:, 0:64], sin_sb[:, ti, 64:128]
                    )
                    nc.vector.tensor_add(kr, tmpk, tmpk2)
                    # V tile (already [s, h]) straight to resident buffer
                    nc.scalar.copy(vres[:, tsl], pskv[:, HD : 2 * HD])

                    # transpose Q tiles to [h, t] (4 into one packed psum half)
                    h = tg[0] % 2
                    tg[0] += 1
                    for n in range(GQ):
                        nc.tensor.transpose(
                            ptx[:, h, n * P : (n + 1) * P],
                            qr[:, n * HD : (n + 1) * HD],
                            ident,
                        )
                    nc.scalar.copy(
                        qtall[:, ti * GQ * P : (ti + 1) * GQ * P], ptx[:, h, :]
                    )
                    h = tg[0] % 2
                    tg[0] += 1
                    nc.tensor.transpose(ptx[:, h, 0:P], kr, ident)
                    nc.scalar.copy(kt[:, tsl], ptx[:, h, 0:P])

                def phase_b(i):
                    jfirst, jcnt = _band(i)
                    w = jcnt * P
                    s0 = jfirst * P
                    ps = [None] * GQ
                    for n in range(GQ):
                        s_ps = s_pool.tile([P, MAXW], FP32, tag="s", name="s_ps")[:, :w]
                        for c0, c1 in _chunks(w):
                            nc.tensor.matmul(
                                s_ps[:, c0:c1],
                                lhsT=qtall[
                                    :, i * GQ * P + n * P : i * GQ * P + (n + 1) * P
                                ],
                                rhs=kt[:, s0 + c0 : s0 + c1],
                                start=True,
                                stop=True,
                            )
                        tf = tb_pool.tile([P, MAXW], FP32, tag="t", name="tf")[:, :w]
                        nc.scalar.activation(
                            tf, s_ps, mybir.ActivationFunctionType.Tanh,
                            scale=tanh_scale,
                        )
                        nc.vector.tensor_add(
                            tf[:, w - P : w], tf[:, w - P : w], maskdiag
                        )
                        if i >= WINDOW // P:
                            nc.vector.tensor_add(tf[:, 0:P], tf[:, 0:P], maskedge)
                        pn = pb_pool.tile([P, MAXW], BF16, tag=f"p{n}", name=f"pn{n}")[
                            :, :w
                        ]
                        r = rb_pool.tile([P, 1], FP32, tag="r", name="r")
                        nc.scalar.activation(
                            pn, tf, mybir.ActivationFunctionType.Exp,
                            scale=SOFT_CAP, accum_out=r,
                        )
                        rr = rb_pool.tile([P, 1], FP32, tag="rr", name="rr")
                        nc.vector.reciprocal(rr, r)
                        nc.vector.tensor_scalar_mul(pn, pn, rr)
                        ps[n] = pn

                    ot = acc_pool.tile([P, GQ * P], FP32, tag="acc", name="ot")
                    for jj in range(jcnt):
                        j = jfirst + jj
                        pt = ptb_pool.tile([P, GQ * P], BF16, tag="pt", name="pt")
                        h = tg[0] % 2
                        tg[0] += 1
                        for n in range(GQ):
                            nc.tensor.transpose(
                                ptx[:, h, n * P : (n + 1) * P],
                                ps[n][:, jj * P : (jj + 1) * P],
                                ident,
                            )
                        nc.vector.tensor_copy(pt, ptx[:, h, :])
                        nc.tensor.matmul(
                            ot,
                            lhsT=vres[:, j * P : (j + 1) * P],
                            rhs=pt,
                            start=(jj == 0),
                            stop=(jj == jcnt - 1),
                        )
                    nc.scalar.copy(
                        enctall[:, i * GQ * P : (i + 1) * GQ * P], ot
                    )

                def phase_c(i):
                    tsl = slice(i * P, (i + 1) * P)
                    osb = oc_pool.tile([P, D], FP32, tag="o", name="osb")
                    for d in range(4):
                        po = acc_pool.tile([P, 512], FP32, tag="acc", name="po")
                        for n in range(GQ):
                            nc.tensor.matmul(
                                po,
                                lhsT=enctall[
                                    :, i * GQ * P + n * P : i * GQ * P + (n + 1) * P
                                ],
                                rhs=wvec_sb[:, n, d * 512 : (d + 1) * 512],
                                start=(n == 0),
                                stop=(n == GQ - 1),
                            )
                        nc.vector.tensor_copy(osb[:, d * 512 : (d + 1) * 512], po)
                    nc.sync.dma_start(out=out_d[tsl, :], in_=osb)

                for step in range(NT + 2):
                    if step < NT:
                        phase_a(step)
                    if 2 <= step:
                        phase_c(step - 2)
                    if 1 <= step <= NT:
                        phase_b(step - 1)

    nc.compile()
    return nc


def _host_inputs(x, segment_pos, wq, wkv, wvec):
    """Build the 8 per-core input maps."""
    import ml_dtypes

    x = np.asarray(x, dtype=np.float32)
    segment_pos = np.asarray(segment_pos)
    wq = np.asarray(wq, dtype=np.float32)
    wkv = np.asarray(wkv, dtype=np.float32)
    wvec = np.asarray(wvec, dtype=np.float32)

    in_maps = []
    for core in range(8):
        b, g = core // 4, core % 4
        xt = np.ascontiguousarray(x[b].T)  # [D, T]
        heads = [wq[4 * g + n] for n in range(GQ)]  # each [D, HD]
        wqkv = np.concatenate(heads + [wkv[0, g], wkv[1, g]], axis=1)  # [D, 768]
        wv = np.ascontiguousarray(wvec[4 * g : 4 * g + 4]).astype(ml_dtypes.bfloat16)
        pos = segment_pos[b].astype(np.float64)  # [T]
        frac = 2.0 * np.arange(HD // 2, dtype=np.float64) / HD
        ts_ = 10000.0 ** frac  # [64]
        ang = pos[:, None] / ts_[None, :]  # [T, 64]
        cos = np.cos(ang).astype(np.float32)
        sin = np.sin(ang).astype(np.float32)
        costab = np.concatenate([cos, cos], axis=1)  # [T, 128]
        sintab = np.concatenate([-sin, sin], axis=1)  # [T, 128]
        in_maps.append(
            {
                "xt": np.ascontiguousarray(xt),
                "wqkv": np.ascontiguousarray(wqkv),
                "wvec": wv,
                "costab": np.ascontiguousarray(costab),
                "sintab": np.ascontiguousarray(sintab),
            }
        )
    return in_maps


def kernel(x, segment_pos, attn_mask, wq, wkv, wvec, _trace=False, _trace_kwargs=None):
    if "nc" not in _COMPILED:
        _COMPILED["nc"] = build_program()
    nc = _COMPILED["nc"]
    in_maps = _host_inputs(x, segment_pos, wq, wkv, wvec)
    kwargs = {}
    if _trace:
        kwargs.update(trace=True)
        if _trace_kwargs:
            kwargs.update(_trace_kwargs)
    res = run_bass_kernel_spmd(nc, in_maps, list(range(8)), **kwargs)
    out = np.empty((B, T, D), dtype=np.float32)
    for b in range(B):
        out[b] = (
            res.results[4 * b + 0]["out"]
            + res.results[4 * b + 1]["out"]
            + res.results[4 * b + 2]["out"]
            + res.results[4 * b + 3]["out"]
        )
    kernel.last_result = res
    return out


# revision 13
# speedup vs baseline: 1.0149x; 1.0149x over previous
"""Trainium2 Bass kernel for GQA sliding-window attention with RoPE + soft-cap.

Problem (hardcoded): B=2, T=2048, D=2048, 16 q-heads / 4 kv-heads, head_dim=128,
WINDOW=1024 (causal sliding window), soft-cap 50.

Sharding: 8 cores = 2 batches x 4-way head-split tensor parallel.
Core c handles batch c//4, q-heads [4g:4g+4] and kv-head g where g = c%4.
Each core emits a partial [T, D] output (sum over its 4 heads); the host sums
the 4 TP partials per batch (the TP all-reduce is done in the unshard step).
"""

import sys

sys.path.insert(0, "/opt/trn_rl_repo")

import math

import numpy as np

import concourse.mybir as mybir
import concourse.tile as tile
from concourse import bacc
from concourse.bass_utils import run_bass_kernel_spmd
from concourse.masks import make_identity

# ---------------------------------------------------------------- constants
B, T, D = 2, 2048, 2048
NH, NKV, HD = 16, 4, 128
GQ = NH // NKV  # 4 q-heads per kv head (= heads per core)
WINDOW = 1024
SOFT_CAP = 50.0
P = 128  # partitions
NT = T // P  # 16 row tiles
ND = D // P  # 16 D chunks
MAXW = 1152  # widest band: 8 full tiles + diagonal tile
MASK_VAL = -1e30

FP32 = mybir.dt.float32
FP32R = mybir.dt.float32r
BF16 = mybir.dt.bfloat16

_COMPILED = {}


def _band(i):
    """Key tiles attended by row tile i: j in [max(0, i-8), i]."""
    jfirst = max(0, i - (WINDOW // P))
    return jfirst, i - jfirst + 1  # first j, tile count (<= 9)


def _chunks(w):
    """Split band width w into matmul chunks aligned to 512-col psum banks."""
    out = []
    c0 = 0
    while c0 < w:
        c1 = min(c0 + 512, w)
        out.append((c0, c1))
        c0 = c1
    return out


def build_program():
    nc = bacc.Bacc(None, target_bir_lowering=False, debug=False)

    xt_d = nc.declare_dram_parameter("xt", [D, T], FP32R, isOutput=False)
    wqkv_d = nc.declare_dram_parameter("wqkv", [D, (GQ + 2) * HD], FP32R, isOutput=False)
    wvec_d = nc.declare_dram_parameter("wvec", [GQ, HD, D], BF16, isOutput=False)
    cos_d = nc.declare_dram_parameter("costab", [T, HD], FP32, isOutput=False)
    sin_d = nc.declare_dram_parameter("sintab", [T, HD], FP32, isOutput=False)
    out_d = nc.declare_dram_parameter("out", [T, D], FP32, isOutput=True)

    tanh_scale = 1.0 / (SOFT_CAP * math.sqrt(HD))

    with tile.TileContext(nc) as tc:
        with (
            tc.tile_pool(name="const", bufs=1) as const,
            tc.tile_pool(name="persist", bufs=1) as persist,
        ):
            ident = const.tile([P, P], BF16)
            make_identity(nc, ident)
            # diag mask: valid iff col <= row (causal within diagonal tile)
            maskdiag = const.tile([P, P], FP32)
            nc.gpsimd.memset(maskdiag, 0.0)
            nc.gpsimd.affine_select(
                out=maskdiag,
                in_=maskdiag,
                compare_op=mybir.AluOpType.is_ge,
                fill=MASK_VAL,
                base=0,
                pattern=[[-1, P]],
                channel_multiplier=1,
            )
            # edge mask: valid iff col > row (window cutoff in oldest tile)
            maskedge = const.tile([P, P], FP32)
            nc.gpsimd.memset(maskedge, 0.0)
            nc.gpsimd.affine_select(
                out=maskedge,
                in_=maskedge,
                compare_op=mybir.AluOpType.is_ge,
                fill=MASK_VAL,
                base=-1,
                pattern=[[1, P]],
                channel_multiplier=-1,
            )

            # resident tensors
            wqkv_sb = persist.tile([P, ND, (GQ + 2) * HD], FP32R)
            wqkv_src = wqkv_d[:].rearrange("(c p) w -> p c w", p=P)
            for d in range(ND):
                nc.sync.dma_start(out=wqkv_sb[:, d, :], in_=wqkv_src[:, d, :])
            cos_sb = persist.tile([P, NT, HD], FP32)
            nc.sync.dma_start(out=cos_sb, in_=cos_d[:].rearrange("(c p) h -> p c h", p=P))
            sin_sb = persist.tile([P, NT, HD], FP32)
            nc.sync.dma_start(out=sin_sb, in_=sin_d[:].rearrange("(c p) h -> p c h", p=P))
            wvec_sb = persist.tile([P, GQ, D], BF16)
            nc.sync.dma_start(
                out=wvec_sb,
                in_=wvec_d[:].rearrange("g (p) d -> p g d", p=P),
            )

            # q^T blocks: (ti, n) block of [h=128, t=128] at cols 512*ti+128*n
            qtall = persist.tile([P, NT * GQ * P], BF16)
            kt = persist.tile([P, T], BF16)
            vres = persist.tile([P, T], BF16)
            # enc^T blocks: same (i, n) block layout as qtall
            enctall = persist.tile([P, NT * GQ * P], BF16)

            # transpose-group half selector (packed psum double-buffer)
            tg = [0]

            with (
                tc.tile_pool(name="xa", bufs=2) as xa_pool,
                tc.tile_pool(name="ra", bufs=3) as ra_pool,
                tc.tile_pool(name="tb", bufs=2) as tb_pool,
                tc.tile_pool(name="pb", bufs=2) as pb_pool,
                tc.tile_pool(name="ptb", bufs=3) as ptb_pool,
                tc.tile_pool(name="rb", bufs=2) as rb_pool,
                tc.tile_pool(name="oc", bufs=2) as oc_pool,
                tc.tile_pool(name="pa", bufs=1, space="PSUM") as pa_pool,
                tc.tile_pool(name="ptx", bufs=1, space="PSUM") as ptx_pool,
                tc.tile_pool(name="sb", bufs=1, space="PSUM") as s_pool,
                tc.tile_pool(name="acc", bufs=2, space="PSUM") as acc_pool,
            ):
                # one bank, manually split into two [P, 512] halves
                ptx = ptx_pool.tile([P, 2, 512], BF16, name="ptx")

                def phase_a(ti):
                    tsl = slice(ti * P, (ti + 1) * P)
                    xt_sb = xa_pool.tile([P, ND, P], FP32R, tag="xt", name="xt_sb")
                    nc.sync.dma_start(
                        out=xt_sb,
                        in_=xt_d[:].rearrange("(c p) t -> p c t", p=P)[:, :, tsl],
                    )
                    psq = pa_pool.tile([P, GQ * HD], FP32, tag="psq", name="psq")
                    pskv = pa_pool.tile([P, 2 * HD], FP32, tag="pskv", name="pskv")
                    for d in range(ND):
                        nc.tensor.matmul(
                            psq,
                            lhsT=xt_sb[:, d, :],
                            rhs=wqkv_sb[:, d, 0 : GQ * HD],
                            start=(d == 0),
                            stop=(d == ND - 1),
                        )
                        nc.tensor.matmul(
                            pskv,
                            lhsT=xt_sb[:, d, :],
                            rhs=wqkv_sb[:, d, GQ * HD : (GQ + 2) * HD],
                            start=(d == 0),
                            stop=(d == ND - 1),
                        )

                    # RoPE, batched across heads (tables hold [cos|cos], [-sin|+sin])
                    qr = ra_pool.tile([P, GQ * HD], BF16, tag="qr", name="qr")
                    psq3 = psq.rearrange("p (n h) -> p n h", n=GQ)
                    qr3 = qr.rearrange("p (n h) -> p n h", n=GQ)
                    cosb = cos_sb[:, ti, :].unsqueeze(1).broadcast_to([P, GQ, HD])
                    sinb0 = sin_sb[:, ti, 0:64].unsqueeze(1).broadcast_to([P, GQ, 64])
                    sinb1 = sin_sb[:, ti, 64:128].unsqueeze(1).broadcast_to([P, GQ, 64])
                    tmpc = ra_pool.tile([P, GQ * HD], FP32, tag="tmpc", name="tmpc")
                    tmps = ra_pool.tile([P, GQ * HD], FP32, tag="tmps", name="tmps")
                    tmpc3 = tmpc.rearrange("p (n h) -> p n h", n=GQ)
                    tmps3 = tmps.rearrange("p (n h) -> p n h", n=GQ)
                    nc.vector.tensor_mul(tmpc3, psq3, cosb)
                    nc.vector.tensor_mul(tmps3[:, :, 0:64], psq3[:, :, 64:128], sinb0)
                    nc.vector.tensor_mul(tmps3[:, :, 64:128], psq3[:, :, 0:64], sinb1)
                    nc.vector.tensor_add(qr3, tmpc3, tmps3)
                    kr = ra_pool.tile([P, HD], BF16, tag="kr", name="kr")
                    tmpk = ra_pool.tile([P, HD], FP32, tag="tmpk", name="tmpk")
                    tmpk2 = ra_pool.tile([P, HD], FP32, tag="tmpk2", name="tmpk2")
                    nc.vector.tensor_mul(tmpk, pskv[:, 0:HD], cos_sb[:, ti, :])
                    nc.vector.tensor_mul(
                        tmpk2[:, 0:64], pskv[:, 64:128], sin_sb[:, ti, 0:64]
                    )
                    nc.vector.tensor_mul(
                        tmpk2[:, 64:128], pskv[:, 0:64], sin_sb[:, ti, 64:128]
                    )
                    nc.vector.tensor_add(kr, tmpk, tmpk2)
                    # V tile (already [s, h]) straight to resident buffer
                    nc.scalar.copy(vres[:, tsl], pskv[:, HD : 2 * HD])

                    # transpose Q tiles to [h, t] (4 into one packed psum half)
                    h = tg[0] % 2
                    tg[0] += 1
                    for n in range(GQ):
                        nc.tensor.transpose(
                            ptx[:, h, n * P : (n + 1) * P],
                            qr[:, n * HD : (n + 1) * HD],
                            ident,
                        )
                    nc.scalar.copy(
                        qtall[:, ti * GQ * P : (ti + 1) * GQ * P], ptx[:, h, :]
                    )
                    h = tg[0] % 2
                    tg[0] += 1
                    nc.tensor.transpose(ptx[:, h, 0:P], kr, ident)
                    nc.scalar.copy(kt[:, tsl], ptx[:, h, 0:P])

                def phase_b(i):
                    jfirst, jcnt = _band(i)
                    w = jcnt * P
                    s0 = jfirst * P
                    ps = [None] * GQ
                    for n in range(GQ):
                        s_ps = s_pool.tile([P, MAXW], FP32, tag="s", name="s_ps")[:, :w]
                        for c0, c1 in _chunks(w):
                            nc.tensor.matmul(
                                s_ps[:, c0:c1],
                                lhsT=qtall[
                                    :, i * GQ * P + n * P : i * GQ * P + (n + 1) * P
                                ],
                                rhs=kt[:, s0 + c0 : s0 + c1],
                                start=True,
                                stop=True,
                            )
                        tf = tb_pool.tile([P, MAXW], FP32, tag="t", name="tf")[:, :w]
                        nc.scalar.activation(
                            tf, s_ps, mybir.ActivationFunctionType.Tanh,
                            scale=tanh_scale,
                        )
                        nc.vector.tensor_add(
                            tf[:, w - P : w], tf[:, w - P : w], maskdiag
                        )
                        if i >= WINDOW // P:
                            nc.vector.tensor_add(tf[:, 0:P], tf[:, 0:P], maskedge)
                        pn = pb_pool.tile([P, MAXW], BF16, tag=f"p{n}", name=f"pn{n}")[
                            :, :w
                        ]
                        r = rb_pool.tile([P, 1], FP32, tag="r", name="r")
                        nc.scalar.activation(
                            pn, tf, mybir.ActivationFunctionType.Exp,
                            scale=SOFT_CAP, accum_out=r,
                        )
                        rr = rb_pool.tile([P, 1], FP32, tag="rr", name="rr")
                        nc.vector.reciprocal(rr, r)
                        nc.vector.tensor_scalar_mul(pn, pn, rr)
                        ps[n] = pn

                    ot = acc_pool.tile([P, GQ * P], FP32, tag="acc", name="ot")
                    for jj in range(jcnt):
                        j = jfirst + jj
                        pt = ptb_pool.tile([P, GQ * P], BF16, tag="pt", name="pt")
                        h = tg[0] % 2
                        tg[0] += 1
                        for n in range(GQ):
                            nc.tensor.transpose(
                                ptx[:, h, n * P : (n + 1) * P],
                                ps[n][:, jj * P : (jj + 1) * P],
                                ident,
                            )
                        nc.vector.tensor_copy(pt, ptx[:, h, :])
                        nc.tensor.matmul(
                            ot,
                            lhsT=vres[:, j * P : (j + 1) * P],
                            rhs=pt,
                            start=(jj == 0),
                            stop=(jj == jcnt - 1),
                        )
                    nc.scalar.copy(
                        enctall[:, i * GQ * P : (i + 1) * GQ * P], ot
                    )

                def phase_c(i):
                    tsl = slice(i * P, (i + 1) * P)
                    osb = oc_pool.tile([P, D], FP32, tag="o", name="osb")
                    for d in range(4):
                        po = acc_pool.tile([P, 512], FP32, tag="acc", name="po")
                        for n in range(GQ):
                            nc.tensor.matmul(
                                po,
                                lhsT=enctall[
                                    :, i * GQ * P + n * P : i * GQ * P + (n + 1) * P
                                ],
                                rhs=wvec_sb[:, n, d * 512 : (d + 1) * 512],
                                start=(n == 0),
                                stop=(n == GQ - 1),
                            )
                        nc.vector.tensor_copy(osb[:, d * 512 : (d + 1) * 512], po)
                    nc.sync.dma_start(out=out_d[tsl, :], in_=osb)

                for step in range(NT + 2):
                    if step < NT:
                        phase_a(step)
                    if 1 <= step <= NT:
                        phase_b(step - 1)
                    if 2 <= step:
                        phase_c(step - 2)

    nc.compile()
    return nc


def _host_inputs(x, segment_pos, wq, wkv, wvec):
    """Build the 8 per-core input maps."""
    import ml_dtypes

    x = np.asarray(x, dtype=np.float32)
    segment_pos = np.asarray(segment_pos)
    wq = np.asarray(wq, dtype=np.float32)
    wkv = np.asarray(wkv, dtype=np.float32)
    wvec = np.asarray(wvec, dtype=np.float32)

    in_maps = []
    for core in range(8):
        b, g = core // 4, core % 4
        xt = np.ascontiguousarray(x[b].T)  # [D, T]
        heads = [wq[4 * g + n] for n in range(GQ)]  # each [D, HD]
        wqkv = np.concatenate(heads + [wkv[0, g], wkv[1, g]], axis=1)  # [D, 768]
        wv = np.ascontiguousarray(wvec[4 * g : 4 * g + 4]).astype(ml_dtypes.bfloat16)
        pos = segment_pos[b].astype(np.float64)  # [T]
        frac = 2.0 * np.arange(HD // 2, dtype=np.float64) / HD
        ts_ = 10000.0 ** frac  # [64]
        ang = pos[:, None] / ts_[None, :]  # [T, 64]
        cos = np.cos(ang).astype(np.float32)
        sin = np.sin(ang).astype(np.float32)
        costab = np.concatenate([cos, cos], axis=1)  # [T, 128]
        sintab = np.concatenate([-sin, sin], axis=1)  # [T, 128]
        in_maps.append(
            {
                "xt": np.ascontiguousarray(xt),
                "wqkv": np.ascontiguousarray(wqkv),
                "wvec": wv,
                "costab": np.ascontiguousarray(costab),
                "sintab": np.ascontiguousarray(sintab),
            }
        )
    return in_maps


def kernel(x, segment_pos, attn_mask, wq, wkv, wvec, _trace=False, _trace_kwargs=None):
    if "nc" not in _COMPILED:
        _COMPILED["nc"] = build_program()
    nc = _COMPILED["nc"]
    in_maps = _host_inputs(x, segment_pos, wq, wkv, wvec)
    kwargs = {}
    if _trace:
        kwargs.update(trace=True)
        if _trace_kwargs:
            kwargs.update(_trace_kwargs)
    res = run_bass_kernel_spmd(nc, in_maps, list(range(8)), **kwargs)
    out = np.empty((B, T, D), dtype=np.float32)
    for b in range(B):
        out[b] = (
            res.results[4 * b + 0]["out"]
            + res.results[4 * b + 1]["out"]
            + res.results[4 * b + 2]["out"]
            + res.results[4 * b + 3]["out"]
        )
    kernel.last_result = res
    return out
